# revision 77
# baseline (speedup 1.0000x reference)
"""Trainium2 Bass kernel for nn_ConvPixelToCapsules.

Reference computation:
  x (16, 256, 1, 20, 20) --conv W (256,1,9,9) stride 2--> votes (16,256,32,8,6,6)
  3 dynamic-routing iterations (softmax over co, weighted sum over ci,
  squash over no, agreement update) -> activation (16, 32, 8, 6, 6)

Sharding: data-parallel over batch, 2 batch elements per core on 8 cores.

Per-core design (v3):
  - Host builds an im2col view of x: xim[k=(ky,kx), plane, pos] bf16,
    so the conv is 36 K=81 bf16 matmuls per 128-plane chunk.
  - votes stored [plane(128 x4 chunks), (pos, no, co)] bf16; `no` in the
    middle keeps innermost AP dims packed for DVE 2x mode everywhere.
  - Iteration-0 routing is uniform, so preact0 = conv(sum_plane x)/32 + bias:
    the host ships xs = sum of im2col over each batch elem's planes and one
    [81,72]x[81,256] matmul produces both batch elems' preact0 up front.
    The iteration-0 distance pass then overlaps the conv chunk loop.
  - Later preacts: ones-vector matmuls into [1,1024] PSUM, staged to
    partition 0, DMA'd into a [36,256] per-b tile (engines can only
    address partitions 0/32/64/96).
  - distances: DMA-broadcast act, big multiply, in-place tree adds over no.
"""

import sys
import functools
import numpy as np

sys.path.insert(0, "/opt/trn_rl_repo")

import concourse.bass as bass  # noqa: E402
import concourse.tile as tile  # noqa: E402
from concourse import mybir  # noqa: E402
from concourse.bass_utils import run_bass_kernel_spmd  # noqa: E402

F32 = mybir.dt.float32
BF16 = mybir.dt.bfloat16

BS, CI, HI, WI = 16, 256, 20, 20
CO, NO, ITERS = 32, 8, 3
KH = KW = 9
K81 = KH * KW               # 81
HOUT = WOUT = 6
POS = HOUT * WOUT           # 36
NCORES = 8
BSH = BS // NCORES          # 2 batch elements per core
PLANES = BSH * CI           # 512
NCHUNK = PLANES // 128      # 4 chunks of 128 planes (b-major)
CONO = CO * NO              # 256
CHW = POS * CONO            # 9216 free elements per chunk of votes (pos, no, co)
HWCO = POS * CO             # 1152 logits free elements per chunk (pos, co)
PP = POS // 2               # 18 pos-pairs
PF = 2 * CONO               # 512 free elements per pos-pair (par, no, co)

Alu = mybir.AluOpType
Act = mybir.ActivationFunctionType
AxX = mybir.AxisListType.X

T0_GP_MD = (1, 3)           # chunks whose t=0 distance multiply runs on gpsimd
T1_GP_MD = (1,)             # same for t=1 (late chunks stay on vector: they
                            # are on the critical path into t=2)


def ap(t, offset, dims):
    """Explicit AP on the same tensor as `t` (an AP), offset in elements."""
    return bass.AP(tensor=t.tensor, offset=t.offset + offset, ap=[list(d) for d in dims])


def _split_excess_waits(nc):
    """Walrus (enable-ldw-opt=false) allows only ONE sync-wait on DMA and
    Matmult/Ldweights pseudo-structs.  Tile sometimes emits 2+ (WAR + WAW).
    Splice a same-engine NoOp carrying the overflow waits in front."""
    import bass_rust

    nid = 0
    for f in nc.m.functions:
        for blk in f.blocks:
            out = []
            changed = False
            for ins in blk.instructions:
                si = ins.sync_info
                if si is not None and len(si.on_wait) > 1:
                    extra = list(si.on_wait)[:-1]
                    keep = list(si.on_wait)[-1:]
                    for w in extra:
                        nop = bass_rust.InstNoOp(name=f"I-waitnop-{nid}")
                        nid += 1
                        nop.engine = ins.engine
                        nop.sync_info = bass_rust.SyncInfo(on_wait=[w], on_update=[])
                        out.append(nop)
                    ins.sync_info = bass_rust.SyncInfo(
                        on_wait=keep, on_update=list(si.on_update))
                    changed = True
                out.append(ins)
            if changed:
                blk.instructions = out


def build_program(split_waits=True):
    nc = bass.Bass("TRN2", target_bir_lowering=False, debug=False)
    xim_d = nc.dram_tensor("xim", [K81, PLANES * POS], BF16, kind="ExternalInput").ap()
    xs_d = nc.dram_tensor("xs", [K81, BSH * POS], BF16, kind="ExternalInput").ap()
    w_d = nc.dram_tensor("w", [K81, CONO], BF16, kind="ExternalInput").ap()
    b_d = nc.dram_tensor("b", [CONO], F32, kind="ExternalInput").ap()
    # out[b, (pos, no, co)] fp32; host transposes to [b, co, no, h, w]
    out_d = nc.dram_tensor("out", [BSH, POS * CONO], F32, kind="ExternalOutput").ap()

    with tile.TileContext(nc) as tc:
        _emit(tc, nc, xim_d, xs_d, w_d, b_d, out_d)
    if split_waits:
        _split_excess_waits(nc)
    return nc


def _emit(tc, nc, xim_d, xs_d, w_d, b_d, out_d):
    import contextlib

    with contextlib.ExitStack() as ctx:
        persist = ctx.enter_context(tc.tile_pool(name="persist", bufs=1))
        dram = ctx.enter_context(tc.tile_pool(name="dram", bufs=1, space="DRAM"))
        rt_ps = ctx.enter_context(tc.tile_pool(name="rt_ps", bufs=1, space="PSUM"))
        stg = ctx.enter_context(tc.tile_pool(name="stg", bufs=1))
        small = ctx.enter_context(tc.tile_pool(name="small", bufs=1))
        big = ctx.enter_context(tc.tile_pool(name="big", bufs=2))
        abcp = ctx.enter_context(tc.tile_pool(name="abcp", bufs=1))

        # ---- persistent tiles ----
        votes = [persist.tile([128, CHW], BF16, name=f"votes{c}") for c in range(NCHUNK)]
        logits = [persist.tile([128, HWCO], F32, name=f"logits{c}") for c in range(NCHUNK)]
        bias72 = persist.tile([BSH * POS, CONO], F32, name="bias72")
        pre36 = [persist.tile([POS, CONO], F32, name=f"pre36_{b}") for b in range(BSH)]
        w_sb = persist.tile([K81, CONO], BF16, name="w_sb")
        xs_sb = persist.tile([K81, BSH * POS], BF16, name="xs_sb")
        ones_bf = nc.const_aps.tensor(1.0, (128, 1), BF16)

        nc.sync.dma_start(out=bias72[:], in_=ap(b_d, 0, [[0, BSH * POS], [1, CONO]]))
        nc.sync.dma_start(out=w_sb[:], in_=ap(w_d, 0, [[CONO, K81], [1, CONO]]))
        nc.sync.dma_start(out=xs_sb[:], in_=ap(xs_d, 0, [[BSH * POS, K81], [1, BSH * POS]]))

        # ---------- helpers ----------
        def squash_fac(preb, P, key):
            """Return the broadcast-view AP of ||s||/(1+||s||^2) over no."""
            sq = small.tile([P, CONO], F32, name="sq", tag=f"sq{key}")
            nc.vector.tensor_tensor(sq[:], preb[:], preb[:], Alu.mult)
            s2 = small.tile([P, CO], F32, name="s2", tag=f"s2{key}")
            nc.vector.tensor_reduce(
                out=s2[:], in_=ap(sq[:], 0, [[CONO, P], [1, CO], [CO, NO]]),
                axis=AxX, op=Alu.add)
            nrm = small.tile([P, CO], F32, name="nrm", tag=f"nrm{key}")
            nc.scalar.activation(out=nrm[:], in_=s2[:], func=Act.Sqrt, scale=1.0)
            d1 = small.tile([P, CO], F32, name="d1", tag=f"d1{key}")
            nc.vector.tensor_scalar_add(d1[:], s2[:], 1.0)
            r1 = small.tile([P, CO], F32, name="r1", tag=f"r1{key}")
            nc.vector.reciprocal(out=r1[:], in_=d1[:])
            fac = small.tile([P, CO], F32, name="fac", tag=f"fac{key}")
            nc.vector.tensor_tensor(fac[:], nrm[:], r1[:], Alu.mult)
            return ap(fac[:], 0, [[CO, P], [0, NO], [1, CO]])

        routes = {}

        def route_chain(c):
            """softmax over co of logits[c] -> normalized route tile.
            Emitted right after logits[c] is final so the next iteration's
            mr loop starts with routes ready."""
            route = small.tile([128, HWCO], BF16, name="route", tag=f"route{c}")
            nc.scalar.activation(out=route[:], in_=logits[c][:],
                                 func=Act.Exp, scale=1.0)
            z = small.tile([128, POS], F32, name="z", tag="z")
            nc.vector.tensor_reduce(
                out=z[:],
                in_=ap(route[:], 0, [[HWCO, 128], [CO, POS], [1, CO]]),
                axis=AxX, op=Alu.add)
            rz = small.tile([128, POS], F32, name="rz", tag="rz")
            nc.vector.reciprocal(out=rz[:], in_=z[:])
            nc.vector.tensor_tensor(
                route[:], route[:],
                ap(rz[:], 0, [[POS, 128], [1, POS], [0, CO]]), Alu.mult)
            return route

        def mr_mult(c):
            """route-weighted votes for chunk c (route from routes[c])."""
            mrt = big.tile([128, CHW], BF16, name="mrt", tag="bigring")
            r_b = ap(routes[c][:], 0, [[HWCO, 128], [CO, POS], [0, NO], [1, CO]])
            nc.vector.tensor_tensor(mrt[:], votes[c][:], r_b, Alu.mult)
            return mrt

        def squash_out(b, final):
            """bias + squash of pre36[b]; DMA out if final, else return the
            broadcast act tile for the distance pass."""
            bias36 = ap(bias72[:], 0, [[CONO, POS], [1, CONO]])
            preb = small.tile([POS, CONO], F32, name="preb",
                              tag="preb0" if b == 0 else "preb1")
            nc.vector.tensor_tensor(preb[:], pre36[b][:], bias36, Alu.add)
            fac_b = squash_fac(preb, POS, "0")
            if final:
                af = small.tile([POS, CONO], F32, name="af", tag="af")
                nc.vector.tensor_tensor(af[:], preb[:], fac_b, Alu.mult)
                nc.sync.dma_start(
                    out=ap(out_d, b * POS * CONO, [[CONO, POS], [1, CONO]]),
                    in_=af[:],
                )
                return None
            ab = small.tile([POS, CONO], BF16, name="ab",
                            tag="ab0" if b == 0 else "ab1")
            nc.vector.tensor_tensor(ab[:], preb[:], fac_b, Alu.mult)
            return bcast_act(ab, b, 0)

        def bcast_act(ab, b, src_off):
            """DMA-bounce act -> DRAM -> [128, CHW] partition-broadcast,
            split across two queues to halve the broadcast latency."""
            adr = dram.tile(list(ab.shape), BF16, name="adr",
                            tag=f"adr{b}_{src_off}_{ab.shape[0]}")
            deng = nc.sync if b == 0 else nc.scalar
            deng.dma_start(out=adr[:], in_=ab[:])
            abc = abcp.tile([128, CHW], BF16, name="abc", tag=f"abc{b}")
            deng.dma_start(out=abc[:],
                           in_=ap(adr[:], src_off, [[0, 128], [1, CHW]]))
            return abc

        def md_tree(c, abc_b, first, gp_set):
            """dist over no for chunk c, accumulated into logits[c]:
            big multiply, then in-place tree adds folding no 8->4->2->1."""
            eng = nc.gpsimd if c in gp_set else nc.vector
            md = big.tile([128, CHW], BF16, name="md", tag="bigring")
            eng.tensor_tensor(md[:], votes[c][:], abc_b[:], Alu.mult)
            v = nc.vector  # trees run 2x on DVE, terribly on gpsimd
            v.tensor_tensor(
                ap(md[:], 0, [[CHW, 128], [CONO, POS], [1, 4 * CO]]),
                ap(md[:], 0, [[CHW, 128], [CONO, POS], [1, 4 * CO]]),
                ap(md[:], 4 * CO, [[CHW, 128], [CONO, POS], [1, 4 * CO]]),
                Alu.add)
            v.tensor_tensor(
                ap(md[:], 0, [[CHW, 128], [CONO, POS], [1, 2 * CO]]),
                ap(md[:], 0, [[CHW, 128], [CONO, POS], [1, 2 * CO]]),
                ap(md[:], 2 * CO, [[CHW, 128], [CONO, POS], [1, 2 * CO]]),
                Alu.add)
            l_in0 = ap(md[:], 0, [[CHW, 128], [CONO, POS], [1, CO]])
            l_in1 = ap(md[:], CO, [[CHW, 128], [CONO, POS], [1, CO]])
            if first:
                v.tensor_tensor(logits[c][:], l_in0, l_in1, Alu.add)
            else:
                # park d in the dead upper half of each pos-block of md
                d = ap(md[:], CONO // 2, [[CHW, 128], [CONO, POS], [1, CO]])
                v.tensor_tensor(d, l_in0, l_in1, Alu.add)
                v.tensor_tensor(logits[c][:], logits[c][:], d, Alu.add)

        def preact_reduce(pair, b):
            """pre36[b][4tq..4tq+3, :] = plane-sums over batch elem b's two
            chunks; [1,1024] PSUM, partition-0 staging, DMA to rows."""
            for tq in range(PP // 2):
                ps1 = rt_ps.tile([1, 2 * PF], F32, name="ps1", tag=f"ps1_{tq % 2}",
                                 bufs=1)
                for half in range(2):
                    t = 2 * tq + half
                    for k in (0, 1):
                        nc.tensor.matmul(
                            ps1[:, half * PF:(half + 1) * PF], ones_bf,
                            ap(pair[k][:], t * PF, [[CHW, 128], [1, PF]]),
                            start=(k == 0), stop=(k == 1),
                            skip_group_check=True,
                        )
                st = stg.tile([1, 2 * PF], F32, name="st", tag=f"st{tq % 2}")
                if tq % 2 == 0:
                    nc.scalar.copy(out=st[:], in_=ps1[:])
                else:
                    nc.vector.tensor_copy(out=st[:], in_=ps1[:])
                deng = (nc.sync, nc.scalar, nc.gpsimd)[tq % 3]
                deng.dma_start(
                    out=ap(pre36[b][:], 4 * tq * CONO, [[CONO, 4], [1, CONO]]),
                    in_=st[:])

        # ---------- t=0 head: preact0 from host-summed planes ----------
        ps0 = rt_ps.tile([BSH * POS, CONO], F32, name="ps0", tag="ps0", bufs=1)
        nc.tensor.matmul(ps0[:], xs_sb[:], w_sb[:], start=True, stop=True)
        preb0 = small.tile([BSH * POS, CONO], F32, name="preb0", tag="preb0")
        nc.vector.scalar_tensor_tensor(
            preb0[:], ps0[:], 1.0 / CO, bias72[:], Alu.mult, Alu.add)
        fac_b0 = squash_fac(preb0, BSH * POS, "0")
        ab0 = small.tile([BSH * POS, CONO], BF16, name="ab0", tag="ab0")
        nc.vector.tensor_tensor(ab0[:], preb0[:], fac_b0, Alu.mult)
        abc0 = {b: bcast_act(ab0, b, b * CHW) for b in range(BSH)}

        # ---------- conv, with the t=0 distance pass riding along ----------
        with tc.tile_pool(name="conv_in", bufs=2) as conv_in, \
             tc.tile_pool(name="conv_ps", bufs=3, space="PSUM") as conv_ps:
            for c in range(NCHUNK):
                im = conv_in.tile([K81, 128 * POS], BF16, name="im", tag="im")
                (nc.sync if c % 2 == 0 else nc.scalar).dma_start(
                    out=im[:],
                    in_=ap(xim_d, c * 128 * POS, [[PLANES * POS, K81], [1, 128 * POS]]),
                )
                for p2 in range(PP):
                    cps = conv_ps.tile([128, PF], F32, name="cps", tag="cps")
                    for par in range(2):
                        pos = 2 * p2 + par
                        lhsT = ap(im[:], pos, [[128 * POS, K81], [POS, 128]])
                        nc.tensor.matmul(
                            cps[:, par * CONO:(par + 1) * CONO], lhsT, w_sb[:],
                            start=True, stop=True,
                        )
                    dst = ap(votes[c][:], p2 * PF, [[CHW, 128], [1, PF]])
                    if p2 % 3 != 1:
                        nc.scalar.copy(out=dst, in_=cps[:])
                    else:
                        nc.vector.tensor_copy(out=dst, in_=cps[:])
                md_tree(c, abc0[c // 2], first=True, gp_set=T0_GP_MD)
                routes[c] = route_chain(c)
                if c == 1:
                    # pull iteration-1's b0 front half into the conv phase:
                    # the PE has slack here, so preact b0 runs concurrently
                    # with the conv of chunks 2-3
                    t1_red0 = [mr_mult(0), mr_mult(1)]
                    preact_reduce(t1_red0, 0)
                    t1_act0 = squash_out(0, False)

        # ---------- iterations 1, 2 ----------
        for t in (1, 2):
            if t == 1:
                acts = [t1_act0]
                red = [mr_mult(2), mr_mult(3)]
                preact_reduce(red, 1)
                # c0/c1 distances are data-ready now; they fill the vector
                # bubble while the PE + stage DMAs finish preact b1
                for c in (0, 1):
                    md_tree(c, acts[0], first=False, gp_set=T1_GP_MD)
                    routes[c] = route_chain(c)
                acts.append(squash_out(1, False))
                for c in (2, 3):
                    md_tree(c, acts[1], first=False, gp_set=T1_GP_MD)
                    routes[c] = route_chain(c)
                continue
            else:
                red = []
                for c in range(NCHUNK):
                    red.append(mr_mult(c))
                    if c % 2 == 1:
                        preact_reduce(red[2 * (c // 2):], c // 2)
                for b in range(BSH):
                    squash_out(b, True)

            if t < ITERS - 1:
                for c in range(NCHUNK):
                    md_tree(c, acts[c // 2], first=False, gp_set=T1_GP_MD)
                    routes[c] = route_chain(c)


@functools.cache
def _program():
    return build_program()


def _host_inputs(x, W, bias):
    """Build per-core input maps: im2col x (bf16) + its per-b plane sums,
    W columns in (no, co) order (bf16), bias flat (no, co) order (f32)."""
    import ml_dtypes

    x = np.asarray(x, dtype=np.float32)
    W = np.asarray(W, dtype=np.float32)
    bias = np.asarray(bias, dtype=np.float32)

    w_t = np.ascontiguousarray(
        W.reshape(CO, NO, K81).transpose(2, 1, 0).reshape(K81, CONO)
    ).astype(ml_dtypes.bfloat16)
    b_flat = np.ascontiguousarray(bias.reshape(CO, NO).T.reshape(CONO))

    in_maps = []
    for i in range(NCORES):
        xs_ = x[i * BSH:(i + 1) * BSH].reshape(PLANES, HI, WI)
        win = np.lib.stride_tricks.sliding_window_view(xs_, (KH, KW), axis=(1, 2))
        win = win[:, ::2, ::2]                    # [plane, oy, ox, ky, kx]
        imcol = np.ascontiguousarray(
            win.transpose(3, 4, 0, 1, 2).reshape(K81, PLANES * POS)
        ).astype(ml_dtypes.bfloat16)
        xsum = (imcol.astype(np.float32)
                .reshape(K81, BSH, CI, POS).sum(axis=2)
                .reshape(K81, BSH * POS)).astype(ml_dtypes.bfloat16)
        in_maps.append({
            "xim": imcol,
            "xs": np.ascontiguousarray(xsum),
            "w": w_t,
            "b": b_flat,
        })
    return in_maps


def kernel(x, W, bias, **_ignored):
    nc = _program()
    in_maps = _host_inputs(x, W, bias)
    res = run_bass_kernel_spmd(nc, in_maps, list(range(NCORES)))
    outs = []
    for i in range(NCORES):
        o = res.results[i]["out"].reshape(BSH, POS, NO, CO)
        outs.append(
            np.ascontiguousarray(o.transpose(0, 3, 2, 1))
            .reshape(BSH, CO, NO, HOUT, WOUT))
    return np.ascontiguousarray(np.concatenate(outs, axis=0))


if __name__ == "__main__":
    xs = np.random.randn(BS, CI, 1, HI, WI).astype(np.float32)
    ws = (np.random.randn(CONO, 1, KH, KW) * 0.05).astype(np.float32)
    bs_ = (np.random.randn(CO, NO, 1, 1) * 0.01).astype(np.float32)
    y = kernel(xs, ws, bs_, quantization_bits=8, quantization_bits_routing=8)
    print(y.shape, y.dtype)


# revision 78
# speedup vs baseline: 1.0113x; 1.0113x over previous
"""Trainium2 Bass kernel for nn_ConvPixelToCapsules.

Reference computation:
  x (16, 256, 1, 20, 20) --conv W (256,1,9,9) stride 2--> votes (16,256,32,8,6,6)
  3 dynamic-routing iterations (softmax over co, weighted sum over ci,
  squash over no, agreement update) -> activation (16, 32, 8, 6, 6)

Sharding: data-parallel over batch, 2 batch elements per core on 8 cores.

Per-core design (v3):
  - Host builds an im2col view of x: xim[k=(ky,kx), plane, pos] bf16,
    so the conv is 36 K=81 bf16 matmuls per 128-plane chunk.
  - votes stored [plane(128 x4 chunks), (pos, no, co)] bf16; `no` in the
    middle keeps innermost AP dims packed for DVE 2x mode everywhere.
  - Iteration-0 routing is uniform, so preact0 = conv(sum_plane x)/32 + bias:
    the host ships xs = sum of im2col over each batch elem's planes and one
    [81,72]x[81,256] matmul produces both batch elems' preact0 up front.
    The iteration-0 distance pass then overlaps the conv chunk loop.
  - Later preacts: ones-vector matmuls into [1,1024] PSUM, staged to
    partition 0, DMA'd into a [36,256] per-b tile (engines can only
    address partitions 0/32/64/96).
  - distances: DMA-broadcast act, big multiply, in-place tree adds over no.
"""

import sys
import functools
import numpy as np

sys.path.insert(0, "/opt/trn_rl_repo")

import concourse.bass as bass  # noqa: E402
import concourse.tile as tile  # noqa: E402
from concourse import mybir  # noqa: E402
from concourse.bass_utils import run_bass_kernel_spmd  # noqa: E402

F32 = mybir.dt.float32
BF16 = mybir.dt.bfloat16

BS, CI, HI, WI = 16, 256, 20, 20
CO, NO, ITERS = 32, 8, 3
KH = KW = 9
K81 = KH * KW               # 81
HOUT = WOUT = 6
POS = HOUT * WOUT           # 36
NCORES = 8
BSH = BS // NCORES          # 2 batch elements per core
PLANES = BSH * CI           # 512
NCHUNK = PLANES // 128      # 4 chunks of 128 planes (b-major)
CONO = CO * NO              # 256
CHW = POS * CONO            # 9216 free elements per chunk of votes (pos, no, co)
HWCO = POS * CO             # 1152 logits free elements per chunk (pos, co)
PP = POS // 2               # 18 pos-pairs
PF = 2 * CONO               # 512 free elements per pos-pair (par, no, co)

Alu = mybir.AluOpType
Act = mybir.ActivationFunctionType
AxX = mybir.AxisListType.X

T0_GP_MD = (1, 3)           # chunks whose t=0 distance multiply runs on gpsimd
T1_GP_MD = (1,)             # same for t=1 (late chunks stay on vector: they
                            # are on the critical path into t=2)


def ap(t, offset, dims):
    """Explicit AP on the same tensor as `t` (an AP), offset in elements."""
    return bass.AP(tensor=t.tensor, offset=t.offset + offset, ap=[list(d) for d in dims])


def _split_excess_waits(nc):
    """Walrus (enable-ldw-opt=false) allows only ONE sync-wait on DMA and
    Matmult/Ldweights pseudo-structs.  Tile sometimes emits 2+ (WAR + WAW).
    Splice a same-engine NoOp carrying the overflow waits in front."""
    import bass_rust

    nid = 0
    for f in nc.m.functions:
        for blk in f.blocks:
            out = []
            changed = False
            for ins in blk.instructions:
                si = ins.sync_info
                if si is not None and len(si.on_wait) > 1:
                    extra = list(si.on_wait)[:-1]
                    keep = list(si.on_wait)[-1:]
                    for w in extra:
                        nop = bass_rust.InstNoOp(name=f"I-waitnop-{nid}")
                        nid += 1
                        nop.engine = ins.engine
                        nop.sync_info = bass_rust.SyncInfo(on_wait=[w], on_update=[])
                        out.append(nop)
                    ins.sync_info = bass_rust.SyncInfo(
                        on_wait=keep, on_update=list(si.on_update))
                    changed = True
                out.append(ins)
            if changed:
                blk.instructions = out


def build_program(split_waits=True):
    nc = bass.Bass("TRN2", target_bir_lowering=False, debug=False)
    xim_d = nc.dram_tensor("xim", [K81, PLANES * POS], BF16, kind="ExternalInput").ap()
    xs_d = nc.dram_tensor("xs", [K81, BSH * POS], BF16, kind="ExternalInput").ap()
    w_d = nc.dram_tensor("w", [K81, CONO], BF16, kind="ExternalInput").ap()
    b_d = nc.dram_tensor("b", [CONO], F32, kind="ExternalInput").ap()
    # out[b, (pos, no, co)] fp32; host transposes to [b, co, no, h, w]
    out_d = nc.dram_tensor("out", [BSH, POS * CONO], F32, kind="ExternalOutput").ap()

    with tile.TileContext(nc) as tc:
        _emit(tc, nc, xim_d, xs_d, w_d, b_d, out_d)
    if split_waits:
        _split_excess_waits(nc)
    return nc


def _emit(tc, nc, xim_d, xs_d, w_d, b_d, out_d):
    import contextlib

    with contextlib.ExitStack() as ctx:
        persist = ctx.enter_context(tc.tile_pool(name="persist", bufs=1))
        dram = ctx.enter_context(tc.tile_pool(name="dram", bufs=1, space="DRAM"))
        rt_ps = ctx.enter_context(tc.tile_pool(name="rt_ps", bufs=1, space="PSUM"))
        stg = ctx.enter_context(tc.tile_pool(name="stg", bufs=1))
        small = ctx.enter_context(tc.tile_pool(name="small", bufs=1))
        big = ctx.enter_context(tc.tile_pool(name="big", bufs=2))
        abcp = ctx.enter_context(tc.tile_pool(name="abcp", bufs=1))

        # ---- persistent tiles ----
        votes = [persist.tile([128, CHW], BF16, name=f"votes{c}") for c in range(NCHUNK)]
        logits = [persist.tile([128, HWCO], F32, name=f"logits{c}") for c in range(NCHUNK)]
        bias72 = persist.tile([BSH * POS, CONO], F32, name="bias72")
        pre36 = [persist.tile([POS, CONO], F32, name=f"pre36_{b}") for b in range(BSH)]
        w_sb = persist.tile([K81, CONO], BF16, name="w_sb")
        xs_sb = persist.tile([K81, BSH * POS], BF16, name="xs_sb")
        ones_bf = nc.const_aps.tensor(1.0, (128, 1), BF16)

        nc.sync.dma_start(out=bias72[:], in_=ap(b_d, 0, [[0, BSH * POS], [1, CONO]]))
        nc.sync.dma_start(out=w_sb[:], in_=ap(w_d, 0, [[CONO, K81], [1, CONO]]))
        nc.sync.dma_start(out=xs_sb[:], in_=ap(xs_d, 0, [[BSH * POS, K81], [1, BSH * POS]]))

        # ---------- helpers ----------
        def squash_fac(preb, P, key):
            """Return the broadcast-view AP of ||s||/(1+||s||^2) over no."""
            sq = small.tile([P, CONO], F32, name="sq", tag=f"sq{key}")
            nc.vector.tensor_tensor(sq[:], preb[:], preb[:], Alu.mult)
            s2 = small.tile([P, CO], F32, name="s2", tag=f"s2{key}")
            nc.vector.tensor_reduce(
                out=s2[:], in_=ap(sq[:], 0, [[CONO, P], [1, CO], [CO, NO]]),
                axis=AxX, op=Alu.add)
            nrm = small.tile([P, CO], F32, name="nrm", tag=f"nrm{key}")
            nc.scalar.activation(out=nrm[:], in_=s2[:], func=Act.Sqrt, scale=1.0)
            d1 = small.tile([P, CO], F32, name="d1", tag=f"d1{key}")
            nc.vector.tensor_scalar_add(d1[:], s2[:], 1.0)
            r1 = small.tile([P, CO], F32, name="r1", tag=f"r1{key}")
            nc.vector.reciprocal(out=r1[:], in_=d1[:])
            fac = small.tile([P, CO], F32, name="fac", tag=f"fac{key}")
            nc.vector.tensor_tensor(fac[:], nrm[:], r1[:], Alu.mult)
            return ap(fac[:], 0, [[CO, P], [0, NO], [1, CO]])

        routes = {}

        def route_chain(c):
            """softmax over co of logits[c] -> normalized route tile.
            Emitted right after logits[c] is final so the next iteration's
            mr loop starts with routes ready."""
            route = small.tile([128, HWCO], BF16, name="route", tag=f"route{c}")
            nc.scalar.activation(out=route[:], in_=logits[c][:],
                                 func=Act.Exp, scale=1.0)
            z = small.tile([128, POS], F32, name="z", tag="z")
            nc.vector.tensor_reduce(
                out=z[:],
                in_=ap(route[:], 0, [[HWCO, 128], [CO, POS], [1, CO]]),
                axis=AxX, op=Alu.add)
            rz = small.tile([128, POS], F32, name="rz", tag="rz")
            nc.vector.reciprocal(out=rz[:], in_=z[:])
            nc.vector.tensor_tensor(
                route[:], route[:],
                ap(rz[:], 0, [[POS, 128], [1, POS], [0, CO]]), Alu.mult)
            return route

        def mr_mult(c):
            """route-weighted votes for chunk c (route from routes[c])."""
            mrt = big.tile([128, CHW], BF16, name="mrt", tag="bigring")
            r_b = ap(routes[c][:], 0, [[HWCO, 128], [CO, POS], [0, NO], [1, CO]])
            nc.vector.tensor_tensor(mrt[:], votes[c][:], r_b, Alu.mult)
            return mrt

        def squash_out(b, final):
            """bias + squash of pre36[b]; DMA out if final, else return the
            broadcast act tile for the distance pass."""
            bias36 = ap(bias72[:], 0, [[CONO, POS], [1, CONO]])
            preb = small.tile([POS, CONO], F32, name="preb",
                              tag="preb0" if b == 0 else "preb1")
            nc.vector.tensor_tensor(preb[:], pre36[b][:], bias36, Alu.add)
            fac_b = squash_fac(preb, POS, "0")
            if final:
                af = small.tile([POS, CONO], F32, name="af", tag="af")
                nc.vector.tensor_tensor(af[:], preb[:], fac_b, Alu.mult)
                nc.sync.dma_start(
                    out=ap(out_d, b * POS * CONO, [[CONO, POS], [1, CONO]]),
                    in_=af[:],
                )
                return None
            ab = small.tile([POS, CONO], BF16, name="ab",
                            tag="ab0" if b == 0 else "ab1")
            nc.vector.tensor_tensor(ab[:], preb[:], fac_b, Alu.mult)
            return bcast_act(ab, b, 0)

        def bcast_act(ab, b, src_off):
            """DMA-bounce act -> DRAM -> [128, CHW] partition-broadcast,
            split across two queues to halve the broadcast latency."""
            adr = dram.tile(list(ab.shape), BF16, name="adr",
                            tag=f"adr{b}_{src_off}_{ab.shape[0]}")
            deng = nc.sync if b == 0 else nc.scalar
            deng.dma_start(out=adr[:], in_=ab[:])
            abc = abcp.tile([128, CHW], BF16, name="abc", tag=f"abc{b}")
            deng.dma_start(out=abc[:],
                           in_=ap(adr[:], src_off, [[0, 128], [1, CHW]]))
            return abc

        def md_tree(c, abc_b, first, gp_set):
            """dist over no for chunk c, accumulated into logits[c]:
            big multiply, then in-place tree adds folding no 8->4->2->1."""
            eng = nc.gpsimd if c in gp_set else nc.vector
            md = big.tile([128, CHW], BF16, name="md", tag="bigring")
            eng.tensor_tensor(md[:], votes[c][:], abc_b[:], Alu.mult)
            v = nc.vector  # trees run 2x on DVE, terribly on gpsimd
            v.tensor_tensor(
                ap(md[:], 0, [[CHW, 128], [CONO, POS], [1, 4 * CO]]),
                ap(md[:], 0, [[CHW, 128], [CONO, POS], [1, 4 * CO]]),
                ap(md[:], 4 * CO, [[CHW, 128], [CONO, POS], [1, 4 * CO]]),
                Alu.add)
            v.tensor_tensor(
                ap(md[:], 0, [[CHW, 128], [CONO, POS], [1, 2 * CO]]),
                ap(md[:], 0, [[CHW, 128], [CONO, POS], [1, 2 * CO]]),
                ap(md[:], 2 * CO, [[CHW, 128], [CONO, POS], [1, 2 * CO]]),
                Alu.add)
            l_in0 = ap(md[:], 0, [[CHW, 128], [CONO, POS], [1, CO]])
            l_in1 = ap(md[:], CO, [[CHW, 128], [CONO, POS], [1, CO]])
            if first:
                v.tensor_tensor(logits[c][:], l_in0, l_in1, Alu.add)
            else:
                # park d in the dead upper half of each pos-block of md
                d = ap(md[:], CONO // 2, [[CHW, 128], [CONO, POS], [1, CO]])
                v.tensor_tensor(d, l_in0, l_in1, Alu.add)
                v.tensor_tensor(logits[c][:], logits[c][:], d, Alu.add)

        def preact_reduce(pair, b):
            """pre36[b][4tq..4tq+3, :] = plane-sums over batch elem b's two
            chunks; [1,1024] PSUM, partition-0 staging, DMA to rows."""
            for tq in range(PP // 2):
                ps1 = rt_ps.tile([1, 2 * PF], F32, name="ps1", tag=f"ps1_{tq % 2}",
                                 bufs=1)
                for half in range(2):
                    t = 2 * tq + half
                    for k in (0, 1):
                        nc.tensor.matmul(
                            ps1[:, half * PF:(half + 1) * PF], ones_bf,
                            ap(pair[k][:], t * PF, [[CHW, 128], [1, PF]]),
                            start=(k == 0), stop=(k == 1),
                            skip_group_check=True,
                        )
                st = stg.tile([1, 2 * PF], F32, name="st", tag=f"st{tq % 2}")
                if tq % 2 == 0:
                    nc.scalar.copy(out=st[:], in_=ps1[:])
                else:
                    nc.vector.tensor_copy(out=st[:], in_=ps1[:])
                deng = (nc.sync, nc.scalar, nc.gpsimd)[tq % 3]
                deng.dma_start(
                    out=ap(pre36[b][:], 4 * tq * CONO, [[CONO, 4], [1, CONO]]),
                    in_=st[:])

        # ---------- t=0 head: preact0 from host-summed planes ----------
        ps0 = rt_ps.tile([BSH * POS, CONO], F32, name="ps0", tag="ps0", bufs=1)
        nc.tensor.matmul(ps0[:], xs_sb[:], w_sb[:], start=True, stop=True)
        preb0 = small.tile([BSH * POS, CONO], F32, name="preb0", tag="preb0")
        nc.vector.scalar_tensor_tensor(
            preb0[:], ps0[:], 1.0 / CO, bias72[:], Alu.mult, Alu.add)
        fac_b0 = squash_fac(preb0, BSH * POS, "0")
        ab0 = small.tile([BSH * POS, CONO], BF16, name="ab0", tag="ab0")
        nc.vector.tensor_tensor(ab0[:], preb0[:], fac_b0, Alu.mult)
        abc0 = {b: bcast_act(ab0, b, b * CHW) for b in range(BSH)}

        # ---------- conv, with the t=0 distance pass riding along ----------
        with tc.tile_pool(name="conv_in", bufs=2) as conv_in, \
             tc.tile_pool(name="conv_ps", bufs=3, space="PSUM") as conv_ps:
            for c in range(NCHUNK):
                im = conv_in.tile([K81, 128 * POS], BF16, name="im", tag="im")
                (nc.sync if c % 2 == 0 else nc.scalar).dma_start(
                    out=im[:],
                    in_=ap(xim_d, c * 128 * POS, [[PLANES * POS, K81], [1, 128 * POS]]),
                )
                for p2 in range(PP):
                    cps = conv_ps.tile([128, PF], F32, name="cps", tag="cps")
                    for par in range(2):
                        pos = 2 * p2 + par
                        lhsT = ap(im[:], pos, [[128 * POS, K81], [POS, 128]])
                        nc.tensor.matmul(
                            cps[:, par * CONO:(par + 1) * CONO], lhsT, w_sb[:],
                            start=True, stop=True,
                        )
                    dst = ap(votes[c][:], p2 * PF, [[CHW, 128], [1, PF]])
                    if p2 % 3 != 1:
                        nc.scalar.copy(out=dst, in_=cps[:])
                    else:
                        nc.vector.tensor_copy(out=dst, in_=cps[:])
                md_tree(c, abc0[c // 2], first=True, gp_set=T0_GP_MD)
                routes[c] = route_chain(c)
                if c == 1:
                    # pull iteration-1's b0 front half into the conv phase:
                    # the PE has slack here, so preact b0 runs concurrently
                    # with the conv of chunks 2-3
                    t1_red0 = [mr_mult(0), mr_mult(1)]
                    preact_reduce(t1_red0, 0)
                    t1_act0 = squash_out(0, False)

        # ---------- iterations 1, 2 ----------
        for t in (1, 2):
            if t == 1:
                acts = [t1_act0]
                red = [mr_mult(2), mr_mult(3)]
                preact_reduce(red, 1)
                acts.append(squash_out(1, False))
            else:
                red = []
                for c in range(NCHUNK):
                    red.append(mr_mult(c))
                    if c % 2 == 1:
                        preact_reduce(red[2 * (c // 2):], c // 2)
                for b in range(BSH):
                    squash_out(b, True)

            if t < ITERS - 1:
                for c in range(NCHUNK):
                    md_tree(c, acts[c // 2], first=False, gp_set=T1_GP_MD)
                    routes[c] = route_chain(c)


@functools.cache
def _program():
    return build_program()


def _host_inputs(x, W, bias):
    """Build per-core input maps: im2col x (bf16) + its per-b plane sums,
    W columns in (no, co) order (bf16), bias flat (no, co) order (f32)."""
    import ml_dtypes

    x = np.asarray(x, dtype=np.float32)
    W = np.asarray(W, dtype=np.float32)
    bias = np.asarray(bias, dtype=np.float32)

    w_t = np.ascontiguousarray(
        W.reshape(CO, NO, K81).transpose(2, 1, 0).reshape(K81, CONO)
    ).astype(ml_dtypes.bfloat16)
    b_flat = np.ascontiguousarray(bias.reshape(CO, NO).T.reshape(CONO))

    in_maps = []
    for i in range(NCORES):
        xs_ = x[i * BSH:(i + 1) * BSH].reshape(PLANES, HI, WI)
        win = np.lib.stride_tricks.sliding_window_view(xs_, (KH, KW), axis=(1, 2))
        win = win[:, ::2, ::2]                    # [plane, oy, ox, ky, kx]
        imcol = np.ascontiguousarray(
            win.transpose(3, 4, 0, 1, 2).reshape(K81, PLANES * POS)
        ).astype(ml_dtypes.bfloat16)
        xsum = (imcol.astype(np.float32)
                .reshape(K81, BSH, CI, POS).sum(axis=2)
                .reshape(K81, BSH * POS)).astype(ml_dtypes.bfloat16)
        in_maps.append({
            "xim": imcol,
            "xs": np.ascontiguousarray(xsum),
            "w": w_t,
            "b": b_flat,
        })
    return in_maps


def kernel(x, W, bias, **_ignored):
    nc = _program()
    in_maps = _host_inputs(x, W, bias)
    res = run_bass_kernel_spmd(nc, in_maps, list(range(NCORES)))
    outs = []
    for i in range(NCORES):
        o = res.results[i]["out"].reshape(BSH, POS, NO, CO)
        outs.append(
            np.ascontiguousarray(o.transpose(0, 3, 2, 1))
            .reshape(BSH, CO, NO, HOUT, WOUT))
    return np.ascontiguousarray(np.concatenate(outs, axis=0))


if __name__ == "__main__":
    xs = np.random.randn(BS, CI, 1, HI, WI).astype(np.float32)
    ws = (np.random.randn(CONO, 1, KH, KW) * 0.05).astype(np.float32)
    bs_ = (np.random.randn(CO, NO, 1, 1) * 0.01).astype(np.float32)
    y = kernel(xs, ws, bs_, quantization_bits=8, quantization_bits_routing=8)
    print(y.shape, y.dtype)


# revision 79
# speedup vs baseline: 1.0570x; 1.0452x over previous
"""Trainium2 Bass kernel for nn_ConvPixelToCapsules.

Reference computation:
  x (16, 256, 1, 20, 20) --conv W (256,1,9,9) stride 2--> votes (16,256,32,8,6,6)
  3 dynamic-routing iterations (softmax over co, weighted sum over ci,
  squash over no, agreement update) -> activation (16, 32, 8, 6, 6)

Sharding: data-parallel over batch, 2 batch elements per core on 8 cores.

Per-core design (v3):
  - Host builds an im2col view of x: xim[k=(ky,kx), plane, pos] bf16,
    so the conv is 36 K=81 bf16 matmuls per 128-plane chunk.
  - votes stored [plane(128 x4 chunks), (pos, no, co)] bf16; `no` in the
    middle keeps innermost AP dims packed for DVE 2x mode everywhere.
  - Iteration-0 routing is uniform, so preact0 = conv(sum_plane x)/32 + bias:
    the host ships xs = sum of im2col over each batch elem's planes and one
    [81,72]x[81,256] matmul produces both batch elems' preact0 up front.
    The iteration-0 distance pass then overlaps the conv chunk loop.
  - Later preacts: ones-vector matmuls into [1,1024] PSUM, staged to
    partition 0, DMA'd into a [36,256] per-b tile (engines can only
    address partitions 0/32/64/96).
  - distances: DMA-broadcast act, big multiply, in-place tree adds over no.
"""

import sys
import functools
import numpy as np

sys.path.insert(0, "/opt/trn_rl_repo")

import concourse.bass as bass  # noqa: E402
import concourse.tile as tile  # noqa: E402
from concourse import mybir  # noqa: E402
from concourse.bass_utils import run_bass_kernel_spmd  # noqa: E402

F32 = mybir.dt.float32
BF16 = mybir.dt.bfloat16

BS, CI, HI, WI = 16, 256, 20, 20
CO, NO, ITERS = 32, 8, 3
KH = KW = 9
K81 = KH * KW               # 81
HOUT = WOUT = 6
POS = HOUT * WOUT           # 36
NCORES = 8
BSH = BS // NCORES          # 2 batch elements per core
PLANES = BSH * CI           # 512
NCHUNK = PLANES // 128      # 4 chunks of 128 planes (b-major)
CONO = CO * NO              # 256
CHW = POS * CONO            # 9216 free elements per chunk of votes (pos, no, co)
HWCO = POS * CO             # 1152 logits free elements per chunk (pos, co)
PP = POS // 2               # 18 pos-pairs
PF = 2 * CONO               # 512 free elements per pos-pair (par, no, co)

Alu = mybir.AluOpType
Act = mybir.ActivationFunctionType
AxX = mybir.AxisListType.X

T0_GP_MD = (1,)             # chunks whose t=0 distance multiply runs on gpsimd
                            # (not 3: a 22us gpsimd op starting at conv-end
                            # would sit on the phase tail; vector does it in 5)
T1_GP_MD = (1,)             # same for t=1 (late chunks stay on vector: they
                            # are on the critical path into t=2)


def ap(t, offset, dims):
    """Explicit AP on the same tensor as `t` (an AP), offset in elements."""
    return bass.AP(tensor=t.tensor, offset=t.offset + offset, ap=[list(d) for d in dims])


def _split_excess_waits(nc):
    """Walrus (enable-ldw-opt=false) allows only ONE sync-wait on DMA and
    Matmult/Ldweights pseudo-structs.  Tile sometimes emits 2+ (WAR + WAW).
    Splice a same-engine NoOp carrying the overflow waits in front."""
    import bass_rust

    nid = 0
    for f in nc.m.functions:
        for blk in f.blocks:
            out = []
            changed = False
            for ins in blk.instructions:
                si = ins.sync_info
                if si is not None and len(si.on_wait) > 1:
                    extra = list(si.on_wait)[:-1]
                    keep = list(si.on_wait)[-1:]
                    for w in extra:
                        nop = bass_rust.InstNoOp(name=f"I-waitnop-{nid}")
                        nid += 1
                        nop.engine = ins.engine
                        nop.sync_info = bass_rust.SyncInfo(on_wait=[w], on_update=[])
                        out.append(nop)
                    ins.sync_info = bass_rust.SyncInfo(
                        on_wait=keep, on_update=list(si.on_update))
                    changed = True
                out.append(ins)
            if changed:
                blk.instructions = out


def build_program(split_waits=True):
    nc = bass.Bass("TRN2", target_bir_lowering=False, debug=False)
    xim_d = nc.dram_tensor("xim", [K81, PLANES * POS], BF16, kind="ExternalInput").ap()
    xs_d = nc.dram_tensor("xs", [K81, BSH * POS], BF16, kind="ExternalInput").ap()
    w_d = nc.dram_tensor("w", [K81, CONO], BF16, kind="ExternalInput").ap()
    b_d = nc.dram_tensor("b", [CONO], F32, kind="ExternalInput").ap()
    # out[b, (pos, no, co)] fp32; host transposes to [b, co, no, h, w]
    out_d = nc.dram_tensor("out", [BSH, POS * CONO], F32, kind="ExternalOutput").ap()

    with tile.TileContext(nc) as tc:
        _emit(tc, nc, xim_d, xs_d, w_d, b_d, out_d)
    if split_waits:
        _split_excess_waits(nc)
    return nc


def _emit(tc, nc, xim_d, xs_d, w_d, b_d, out_d):
    import contextlib

    with contextlib.ExitStack() as ctx:
        persist = ctx.enter_context(tc.tile_pool(name="persist", bufs=1))
        dram = ctx.enter_context(tc.tile_pool(name="dram", bufs=1, space="DRAM"))
        rt_ps = ctx.enter_context(tc.tile_pool(name="rt_ps", bufs=1, space="PSUM"))
        stg = ctx.enter_context(tc.tile_pool(name="stg", bufs=1))
        small = ctx.enter_context(tc.tile_pool(name="small", bufs=1))
        big = ctx.enter_context(tc.tile_pool(name="big", bufs=2))
        abcp = ctx.enter_context(tc.tile_pool(name="abcp", bufs=1))

        # ---- persistent tiles ----
        votes = [persist.tile([128, CHW], BF16, name=f"votes{c}") for c in range(NCHUNK)]
        logits = [persist.tile([128, HWCO], F32, name=f"logits{c}") for c in range(NCHUNK)]
        bias72 = persist.tile([BSH * POS, CONO], F32, name="bias72")
        pre36 = [persist.tile([POS, CONO], F32, name=f"pre36_{b}") for b in range(BSH)]
        w_sb = persist.tile([K81, CONO], BF16, name="w_sb")
        xs_sb = persist.tile([K81, BSH * POS], BF16, name="xs_sb")
        ones_bf = nc.const_aps.tensor(1.0, (128, 1), BF16)

        nc.sync.dma_start(out=bias72[:], in_=ap(b_d, 0, [[0, BSH * POS], [1, CONO]]))
        nc.sync.dma_start(out=w_sb[:], in_=ap(w_d, 0, [[CONO, K81], [1, CONO]]))
        nc.sync.dma_start(out=xs_sb[:], in_=ap(xs_d, 0, [[BSH * POS, K81], [1, BSH * POS]]))

        # ---------- helpers ----------
        def squash_fac(preb, P, key):
            """Return the broadcast-view AP of ||s||/(1+||s||^2) over no."""
            sq = small.tile([P, CONO], F32, name="sq", tag=f"sq{key}")
            nc.vector.tensor_tensor(sq[:], preb[:], preb[:], Alu.mult)
            s2 = small.tile([P, CO], F32, name="s2", tag=f"s2{key}")
            nc.vector.tensor_reduce(
                out=s2[:], in_=ap(sq[:], 0, [[CONO, P], [1, CO], [CO, NO]]),
                axis=AxX, op=Alu.add)
            nrm = small.tile([P, CO], F32, name="nrm", tag=f"nrm{key}")
            nc.scalar.activation(out=nrm[:], in_=s2[:], func=Act.Sqrt, scale=1.0)
            d1 = small.tile([P, CO], F32, name="d1", tag=f"d1{key}")
            nc.vector.tensor_scalar_add(d1[:], s2[:], 1.0)
            r1 = small.tile([P, CO], F32, name="r1", tag=f"r1{key}")
            nc.vector.reciprocal(out=r1[:], in_=d1[:])
            fac = small.tile([P, CO], F32, name="fac", tag=f"fac{key}")
            nc.vector.tensor_tensor(fac[:], nrm[:], r1[:], Alu.mult)
            return ap(fac[:], 0, [[CO, P], [0, NO], [1, CO]])

        routes = {}

        def route_chain(c):
            """softmax over co of logits[c] -> normalized route tile.
            Emitted right after logits[c] is final so the next iteration's
            mr loop starts with routes ready."""
            route = small.tile([128, HWCO], BF16, name="route", tag=f"route{c}")
            nc.scalar.activation(out=route[:], in_=logits[c][:],
                                 func=Act.Exp, scale=1.0)
            z = small.tile([128, POS], F32, name="z", tag="z")
            nc.vector.tensor_reduce(
                out=z[:],
                in_=ap(route[:], 0, [[HWCO, 128], [CO, POS], [1, CO]]),
                axis=AxX, op=Alu.add)
            rz = small.tile([128, POS], F32, name="rz", tag="rz")
            nc.vector.reciprocal(out=rz[:], in_=z[:])
            nc.vector.tensor_tensor(
                route[:], route[:],
                ap(rz[:], 0, [[POS, 128], [1, POS], [0, CO]]), Alu.mult)
            return route

        def mr_mult(c):
            """route-weighted votes for chunk c (route from routes[c])."""
            mrt = big.tile([128, CHW], BF16, name="mrt", tag="bigring")
            r_b = ap(routes[c][:], 0, [[HWCO, 128], [CO, POS], [0, NO], [1, CO]])
            nc.vector.tensor_tensor(mrt[:], votes[c][:], r_b, Alu.mult)
            return mrt

        def squash_out(b, final):
            """bias + squash of pre36[b]; DMA out if final, else return the
            broadcast act tile for the distance pass."""
            bias36 = ap(bias72[:], 0, [[CONO, POS], [1, CONO]])
            preb = small.tile([POS, CONO], F32, name="preb",
                              tag="preb0" if b == 0 else "preb1")
            nc.vector.tensor_tensor(preb[:], pre36[b][:], bias36, Alu.add)
            fac_b = squash_fac(preb, POS, "0")
            if final:
                af = small.tile([POS, CONO], F32, name="af", tag="af")
                nc.vector.tensor_tensor(af[:], preb[:], fac_b, Alu.mult)
                nc.sync.dma_start(
                    out=ap(out_d, b * POS * CONO, [[CONO, POS], [1, CONO]]),
                    in_=af[:],
                )
                return None
            ab = small.tile([POS, CONO], BF16, name="ab",
                            tag="ab0" if b == 0 else "ab1")
            nc.vector.tensor_tensor(ab[:], preb[:], fac_b, Alu.mult)
            return bcast_act(ab, b, 0)

        def bcast_act(ab, b, src_off):
            """DMA-bounce act -> DRAM -> [128, CHW] partition-broadcast,
            split across two queues to halve the broadcast latency."""
            adr = dram.tile(list(ab.shape), BF16, name="adr",
                            tag=f"adr{b}_{src_off}_{ab.shape[0]}")
            deng = nc.sync if b == 0 else nc.scalar
            deng.dma_start(out=adr[:], in_=ab[:])
            abc = abcp.tile([128, CHW], BF16, name="abc", tag=f"abc{b}")
            deng.dma_start(out=abc[:],
                           in_=ap(adr[:], src_off, [[0, 128], [1, CHW]]))
            return abc

        def md_tree(c, abc_b, first, gp_set):
            """dist over no for chunk c, accumulated into logits[c]:
            big multiply, then in-place tree adds folding no 8->4->2->1."""
            eng = nc.gpsimd if c in gp_set else nc.vector
            md = big.tile([128, CHW], BF16, name="md", tag="bigring")
            eng.tensor_tensor(md[:], votes[c][:], abc_b[:], Alu.mult)
            v = nc.vector  # trees run 2x on DVE, terribly on gpsimd
            v.tensor_tensor(
                ap(md[:], 0, [[CHW, 128], [CONO, POS], [1, 4 * CO]]),
                ap(md[:], 0, [[CHW, 128], [CONO, POS], [1, 4 * CO]]),
                ap(md[:], 4 * CO, [[CHW, 128], [CONO, POS], [1, 4 * CO]]),
                Alu.add)
            v.tensor_tensor(
                ap(md[:], 0, [[CHW, 128], [CONO, POS], [1, 2 * CO]]),
                ap(md[:], 0, [[CHW, 128], [CONO, POS], [1, 2 * CO]]),
                ap(md[:], 2 * CO, [[CHW, 128], [CONO, POS], [1, 2 * CO]]),
                Alu.add)
            l_in0 = ap(md[:], 0, [[CHW, 128], [CONO, POS], [1, CO]])
            l_in1 = ap(md[:], CO, [[CHW, 128], [CONO, POS], [1, CO]])
            if first:
                v.tensor_tensor(logits[c][:], l_in0, l_in1, Alu.add)
            else:
                # park d in the dead upper half of each pos-block of md
                d = ap(md[:], CONO // 2, [[CHW, 128], [CONO, POS], [1, CO]])
                v.tensor_tensor(d, l_in0, l_in1, Alu.add)
                v.tensor_tensor(logits[c][:], logits[c][:], d, Alu.add)

        def preact_reduce(pair, b):
            """pre36[b][4tq..4tq+3, :] = plane-sums over batch elem b's two
            chunks; [1,1024] PSUM, partition-0 staging, DMA to rows."""
            for tq in range(PP // 2):
                ps1 = rt_ps.tile([1, 2 * PF], F32, name="ps1", tag=f"ps1_{tq % 2}",
                                 bufs=1)
                for half in range(2):
                    t = 2 * tq + half
                    for k in (0, 1):
                        nc.tensor.matmul(
                            ps1[:, half * PF:(half + 1) * PF], ones_bf,
                            ap(pair[k][:], t * PF, [[CHW, 128], [1, PF]]),
                            start=(k == 0), stop=(k == 1),
                            skip_group_check=True,
                        )
                st = stg.tile([1, 2 * PF], F32, name="st", tag=f"st{tq % 2}")
                if tq % 2 == 0:
                    nc.scalar.copy(out=st[:], in_=ps1[:])
                else:
                    nc.vector.tensor_copy(out=st[:], in_=ps1[:])
                deng = (nc.sync, nc.scalar, nc.gpsimd)[tq % 3]
                deng.dma_start(
                    out=ap(pre36[b][:], 4 * tq * CONO, [[CONO, 4], [1, CONO]]),
                    in_=st[:])

        # ---------- t=0 head: preact0 from host-summed planes ----------
        ps0 = rt_ps.tile([BSH * POS, CONO], F32, name="ps0", tag="ps0", bufs=1)
        nc.tensor.matmul(ps0[:], xs_sb[:], w_sb[:], start=True, stop=True)
        preb0 = small.tile([BSH * POS, CONO], F32, name="preb0", tag="preb0")
        nc.vector.scalar_tensor_tensor(
            preb0[:], ps0[:], 1.0 / CO, bias72[:], Alu.mult, Alu.add)
        fac_b0 = squash_fac(preb0, BSH * POS, "0")
        ab0 = small.tile([BSH * POS, CONO], BF16, name="ab0", tag="ab0")
        nc.vector.tensor_tensor(ab0[:], preb0[:], fac_b0, Alu.mult)
        abc0 = {b: bcast_act(ab0, b, b * CHW) for b in range(BSH)}

        # ---------- conv, with the t=0 distance pass riding along ----------
        with tc.tile_pool(name="conv_in", bufs=2) as conv_in, \
             tc.tile_pool(name="conv_ps", bufs=3, space="PSUM") as conv_ps:
            for c in range(NCHUNK):
                im = conv_in.tile([K81, 128 * POS], BF16, name="im", tag="im")
                (nc.sync if c % 2 == 0 else nc.scalar).dma_start(
                    out=im[:],
                    in_=ap(xim_d, c * 128 * POS, [[PLANES * POS, K81], [1, 128 * POS]]),
                )
                for p2 in range(PP):
                    cps = conv_ps.tile([128, PF], F32, name="cps", tag="cps")
                    for par in range(2):
                        pos = 2 * p2 + par
                        lhsT = ap(im[:], pos, [[128 * POS, K81], [POS, 128]])
                        nc.tensor.matmul(
                            cps[:, par * CONO:(par + 1) * CONO], lhsT, w_sb[:],
                            start=True, stop=True,
                        )
                    dst = ap(votes[c][:], p2 * PF, [[CHW, 128], [1, PF]])
                    if p2 % 3 != 1:
                        nc.scalar.copy(out=dst, in_=cps[:])
                    else:
                        nc.vector.tensor_copy(out=dst, in_=cps[:])
                md_tree(c, abc0[c // 2], first=True, gp_set=T0_GP_MD)
                routes[c] = route_chain(c)
                if c == 1:
                    # pull iteration-1's b0 front half into the conv phase:
                    # the PE has slack here, so preact b0 runs concurrently
                    # with the conv of chunks 2-3
                    t1_red0 = [mr_mult(0), mr_mult(1)]
                    preact_reduce(t1_red0, 0)
                    t1_act0 = squash_out(0, False)

        # ---------- iterations 1, 2 ----------
        for t in (1, 2):
            if t == 1:
                acts = [t1_act0]
                red = [mr_mult(2), mr_mult(3)]
                preact_reduce(red, 1)
                acts.append(squash_out(1, False))
            else:
                red = []
                for c in range(NCHUNK):
                    red.append(mr_mult(c))
                    if c % 2 == 1:
                        preact_reduce(red[2 * (c // 2):], c // 2)
                for b in range(BSH):
                    squash_out(b, True)

            if t < ITERS - 1:
                for c in range(NCHUNK):
                    md_tree(c, acts[c // 2], first=False, gp_set=T1_GP_MD)
                    routes[c] = route_chain(c)


@functools.cache
def _program():
    return build_program()


def _host_inputs(x, W, bias):
    """Build per-core input maps: im2col x (bf16) + its per-b plane sums,
    W columns in (no, co) order (bf16), bias flat (no, co) order (f32)."""
    import ml_dtypes

    x = np.asarray(x, dtype=np.float32)
    W = np.asarray(W, dtype=np.float32)
    bias = np.asarray(bias, dtype=np.float32)

    w_t = np.ascontiguousarray(
        W.reshape(CO, NO, K81).transpose(2, 1, 0).reshape(K81, CONO)
    ).astype(ml_dtypes.bfloat16)
    b_flat = np.ascontiguousarray(bias.reshape(CO, NO).T.reshape(CONO))

    in_maps = []
    for i in range(NCORES):
        xs_ = x[i * BSH:(i + 1) * BSH].reshape(PLANES, HI, WI)
        win = np.lib.stride_tricks.sliding_window_view(xs_, (KH, KW), axis=(1, 2))
        win = win[:, ::2, ::2]                    # [plane, oy, ox, ky, kx]
        imcol = np.ascontiguousarray(
            win.transpose(3, 4, 0, 1, 2).reshape(K81, PLANES * POS)
        ).astype(ml_dtypes.bfloat16)
        xsum = (imcol.astype(np.float32)
                .reshape(K81, BSH, CI, POS).sum(axis=2)
                .reshape(K81, BSH * POS)).astype(ml_dtypes.bfloat16)
        in_maps.append({
            "xim": imcol,
            "xs": np.ascontiguousarray(xsum),
            "w": w_t,
            "b": b_flat,
        })
    return in_maps


def kernel(x, W, bias, **_ignored):
    nc = _program()
    in_maps = _host_inputs(x, W, bias)
    res = run_bass_kernel_spmd(nc, in_maps, list(range(NCORES)))
    outs = []
    for i in range(NCORES):
        o = res.results[i]["out"].reshape(BSH, POS, NO, CO)
        outs.append(
            np.ascontiguousarray(o.transpose(0, 3, 2, 1))
            .reshape(BSH, CO, NO, HOUT, WOUT))
    return np.ascontiguousarray(np.concatenate(outs, axis=0))


if __name__ == "__main__":
    xs = np.random.randn(BS, CI, 1, HI, WI).astype(np.float32)
    ws = (np.random.randn(CONO, 1, KH, KW) * 0.05).astype(np.float32)
    bs_ = (np.random.randn(CO, NO, 1, 1) * 0.01).astype(np.float32)
    y = kernel(xs, ws, bs_, quantization_bits=8, quantization_bits_routing=8)
    print(y.shape, y.dtype)
